# revision 33
# baseline (speedup 1.0000x reference)
"""BotRGCN forward on 8 TRN2 NeuronCores (Bass/Tile SPMD kernel).

Strategy (self-contained; shapes hardcoded for nn_BotRGCN1):
  - Nodes sharded 8-way (6272/core, N padded 50000->50176).
  - f16 on-chip compute everywhere (fp32 PSUM accumulation).
  - RGCN: aggregate-then-transform.  Edge messages gathered with
    dma_gather (f16 rows, 256B, src-sorted for HBM row locality) in
    256-token chunks on 4 SWDGE queues; segment-summed on the
    TensorEngine via per-block one-hot matmuls with the 1/deg weight
    fused into the one-hot build (DVE is_equal*mult), PSUM accumulated
    per (dst-tile, relation) across BOTH src halves, so the W_r
    transform runs once per tile.
  - Scalar engine (ACT) does all PSUM drains, bias adds and LeakyReLU.
  - Boundary exchange: AllGather in 5 row-chunks interleaved with
    compute; xf rows are chunk-major so each chunk lands contiguously.
  - Edges partitioned by dst core, grouped by (dst-tile, src-half,
    relation); block schedule is the max over cores (SPMD).
"""
import numpy as np

N = 50000
M = 8
L = 6272            # nodes per core (N padded to 50176)
NPAD = M * L
D = 128
DDES = 768
R = 5
TW = 512            # dst tile width
NT = 13             # ceil(L/TW): 12 full + 1 of 128
SPLIT = 32768       # src half split (on remapped xf rows) for int16 idx
CHUNK = 256         # tokens per dma_gather
SLOPE = 0.01
NQUEUES = 4

# AllGather row-chunks (within-core row ranges); xf layout is
# chunk-major: chunk k holds [M, b-a, D] contiguously at row M*a.
AGCH = [(0, 3072), (3072, L)]
AG_AFTER_TILE = {5: 0, 12: 1}  # tile idx -> chunk idx

_LAST = {}          # exec stats for test harness


def _tile_w(t):
    return min(TW, L - t * TW)


def _xf_row(node):
    """Remapped row of a (global) node id in the chunk-major xf layout."""
    m, loc = node // L, node % L
    a_arr = np.array([a for a, b in AGCH])
    b_arr = np.array([b for a, b in AGCH])
    k = np.searchsorted(b_arr, loc, side="right")
    a, w = a_arr[k], (b_arr - a_arr)[k]
    return M * a + m * w + (loc - a)


def _prep_edges(edge_index, edge_type):
    """Per-core token streams + shared (max-over-cores) block schedule.

    Group order is (dst-tile t, src-half h, relation r) so each tile's
    5 PSUM accumulators span both halves and are transformed once.
    """
    src = np.asarray(edge_index[0], dtype=np.int64)
    dst = np.asarray(edge_index[1], dtype=np.int64)
    et = np.asarray(edge_type, dtype=np.int64)

    xrow = _xf_row(src)
    h_all = (xrow >= SPLIT).astype(np.int64)
    idx16_all = (xrow - h_all * SPLIT).astype(np.int16)

    core = dst // L
    dloc = dst % L
    t_all = dloc // TW
    doff_all = (dloc - t_all * TW).astype(np.float32)

    NG = NT * 2 * R
    per_core_groups = []
    for m in range(M):
        sel = np.nonzero(core == m)[0]
        dl, r = dloc[sel], et[sel]
        cnt = np.zeros((R, L), np.float32)
        np.add.at(cnt, (r, dl), 1.0)
        rcp = 1.0 / np.maximum(cnt, 1.0)
        key = (t_all[sel] * 2 + h_all[sel]) * R + r
        order = np.argsort(key, kind="stable")
        sel = sel[order]
        key = key[order]
        bounds = np.searchsorted(key, np.arange(NG + 1))
        groups = {}
        for gk in range(NG):
            a, b = bounds[gk], bounds[gk + 1]
            gt, gh, gr = gk // (2 * R), (gk // R) % 2, gk % R
            e = sel[a:b]
            # sort by src address for HBM row locality in the gather
            e = e[np.argsort(idx16_all[e], kind="stable")]
            groups[(gt, gh, gr)] = (
                idx16_all[e], doff_all[e],
                rcp[et[e], dloc[e]].astype(np.float32))
        per_core_groups.append(groups)

    nblk = {}
    for gt in range(NT):
        for gh in range(2):
            for gr in range(R):
                mx = max(len(per_core_groups[m][(gt, gh, gr)][0])
                         for m in range(M))
                nblk[(gt, gh, gr)] = max(1, -(-mx // 128))

    TTOT = 128 * sum(nblk.values())
    NBLK = TTOT // 128
    gidx_all = np.zeros((M, TTOT), np.int16)
    gdst_all = np.full((M, TTOT), -1.0, np.float32)
    grcp_all = np.zeros((M, TTOT), np.float32)
    pos = 0
    sched = []   # per block: (h, t, r, first, last)
    chunks = []  # (half, tok_start, ntok)
    for gt in range(NT):
        for gh in range(2):
            run_start = pos
            for gr in range(R):
                nb = nblk[(gt, gh, gr)]
                for m in range(M):
                    gi, gd, gc = per_core_groups[m][(gt, gh, gr)]
                    n = len(gi)
                    gidx_all[m, pos:pos + n] = gi
                    gdst_all[m, pos:pos + n] = gd
                    grcp_all[m, pos:pos + n] = gc
                for j in range(nb):
                    first = (gh == 0 and j == 0)
                    last = (gh == 1 and j == nb - 1)
                    sched.append((gh, gt, gr, first, last))
                pos += nb * 128
            run = pos - run_start
            s0 = run_start
            while s0 < pos:
                n = min(CHUNK, pos - s0)
                chunks.append((gh, s0, n))
                s0 += n
    assert pos == TTOT

    gidx_w = np.tile(
        gidx_all.reshape(M, TTOT // 16, 16).transpose(0, 2, 1), (1, 8, 1)
    ).copy()                                            # [M, 128, TTOT//16]
    gdst_w = gdst_all.reshape(M, NBLK, 128).transpose(0, 2, 1).copy()
    grcp_w = grcp_all.reshape(M, NBLK, 128).transpose(0, 2, 1).copy()
    return gidx_w, gdst_w, grcp_w, sched, chunks, TTOT, NBLK


def _build(sched, chunks, TTOT, NBLK):
    from concourse import bacc, tile, mybir

    nc = bacc.Bacc("TRN2", target_bir_lowering=False, debug=False,
                   num_devices=M, num_swdge_queues=NQUEUES)
    f32, i16 = mybir.dt.float32, mybir.dt.int16
    f16 = mybir.dt.float16
    Alu = mybir.AluOpType
    Act = mybir.ActivationFunctionType

    desT_d = nc.dram_tensor("desT", [128, 6, L], f16, kind="ExternalInput")
    gidx_d = nc.dram_tensor("gidx", [128, TTOT // 16], i16, kind="ExternalInput")
    gdst_d = nc.dram_tensor("gdst", [128, NBLK], f32, kind="ExternalInput")
    grcp_d = nc.dram_tensor("grcp", [128, NBLK], f32, kind="ExternalInput")
    wdes_d = nc.dram_tensor("wdes", [DDES, D], f16, kind="ExternalInput")
    win_d = nc.dram_tensor("win", [D, D], f16, kind="ExternalInput")
    wroot_d = nc.dram_tensor("wroot", [D, D], f16, kind="ExternalInput")
    wrel_d = nc.dram_tensor("wrel", [R, D, D], f16, kind="ExternalInput")
    wout1_d = nc.dram_tensor("wout1", [D, D], f16, kind="ExternalInput")
    wout2_d = nc.dram_tensor("wout2", [D, 2], f16, kind="ExternalInput")
    bias_d = nc.dram_tensor("bias", [D, 4], f32, kind="ExternalInput")
    bout2_d = nc.dram_tensor("bout2", [2, 1], f32, kind="ExternalInput")
    out_d = nc.dram_tensor("out", [2, L], f32, kind="ExternalOutput")

    y_loc = [nc.dram_tensor(f"y_loc{i}", [L, D], f16) for i in range(2)]
    xf = [nc.dram_tensor(f"xf{i}", [NPAD, D], f16, addr_space="Shared")
          for i in range(2)]

    iota = nc.inline_tensor(
        np.broadcast_to(np.arange(TW, dtype=np.float16), (128, TW)).copy(),
        "iota")
    ident = nc.inline_tensor(np.eye(128, dtype=np.float16), "ident")

    with tile.TileContext(nc) as tc:
        with (
            tc.tile_pool(name="cst", bufs=1) as cst,
            tc.tile_pool(name="big", bufs=2) as big,
            tc.tile_pool(name="wk", bufs=4) as wk,
            tc.tile_pool(name="ps", bufs=1, space="PSUM") as psp,
        ):
            # ---- constants to SBUF ----
            iota_sb = cst.tile([128, TW], f16)
            nc.sync.dma_start(out=iota_sb[:], in_=iota[:])
            ident_sb = cst.tile([128, 128], f16)
            nc.sync.dma_start(out=ident_sb[:], in_=ident[:])
            gidx_sb = cst.tile([128, TTOT // 16], i16)
            nc.sync.dma_start(out=gidx_sb[:], in_=gidx_d[:])
            gdst_sb = cst.tile([128, NBLK], f32)
            nc.sync.dma_start(out=gdst_sb[:], in_=gdst_d[:])
            grcp_sb = cst.tile([128, NBLK], f32)
            nc.sync.dma_start(out=grcp_sb[:], in_=grcp_d[:])
            wdes_sb = cst.tile([128, 6, D], f16)
            for k in range(6):
                nc.sync.dma_start(out=wdes_sb[:, k, :],
                                  in_=wdes_d[k * 128:(k + 1) * 128, :])
            win_sb = cst.tile([128, D], f16)
            nc.sync.dma_start(out=win_sb[:], in_=win_d[:])
            wroot_sb = cst.tile([128, D], f16)
            nc.sync.dma_start(out=wroot_sb[:], in_=wroot_d[:])
            wrel_sb = cst.tile([128, R, D], f16)
            for r in range(R):
                nc.sync.dma_start(out=wrel_sb[:, r, :], in_=wrel_d[r])
            wout1_sb = cst.tile([128, D], f16)
            nc.sync.dma_start(out=wout1_sb[:], in_=wout1_d[:])
            wout2_sb = cst.tile([128, 2], f16)
            nc.sync.dma_start(out=wout2_sb[:], in_=wout2_d[:])
            bias_sb = cst.tile([128, 4], f32)
            nc.sync.dma_start(out=bias_sb[:], in_=bias_d[:])
            bout2_sb = cst.tile([2, 1], f32)
            nc.sync.dma_start(out=bout2_sb[:], in_=bout2_d[:])

            def transpose_store(src_ap, t, w, yl):
                """feature-major f16 [128, w] -> yl rows (node-major)."""
                for b in range(-(-w // 128)):
                    bw = min(128, w - b * 128)
                    trp = psp.tile([128, 128], f16, tag="tr", bufs=2)
                    nc.tensor.transpose(
                        trp[:bw, :], src_ap[:, b * 128:b * 128 + bw],
                        ident_sb[:])
                    ynm = wk.tile([128, D], f16, tag="ynm", bufs=4)
                    nc.scalar.activation(ynm[:bw, :], trp[:bw, :], Act.Copy)
                    r0 = t * TW + b * 128
                    nc.sync.dma_start(out=yl[r0:r0 + bw, :], in_=ynm[:bw, :])

            def ag_chunk(k, yl, xfull):
                a, b = AGCH[k]
                nc.gpsimd.collective_compute(
                    "AllGather", mybir.AluOpType.bypass,
                    replica_groups=[list(range(M))],
                    ins=[yl[a:b, :]],
                    outs=[xfull[M * a:M * b, :]])


            # ================= MLP =================
            x1T = big.tile([128, L], f16, tag="bigT", name="x1T")
            for c in range(NT):
                w = _tile_w(c)
                dt6 = wk.tile([128, 6, TW], f16, tag="des", bufs=2)
                nc.sync.dma_start(out=dt6[:, :, :w],
                                  in_=desT_d[:, :, c * TW:c * TW + w])
                ps = psp.tile([128, TW], f32, tag="out")
                for k in range(6):
                    nc.tensor.matmul(ps[:, :w], wdes_sb[:, k, :],
                                     dt6[:, k, :w],
                                     start=(k == 0), stop=(k == 5))
                x0c = wk.tile([128, TW], f16, tag="x0c")
                nc.scalar.activation(x0c[:, :w], ps[:, :w], Act.Lrelu,
                                     bias=bias_sb[:, 0:1], alpha=SLOPE)
                ps2 = psp.tile([128, TW], f32, tag="out")
                nc.tensor.matmul(ps2[:, :w], win_sb[:], x0c[:, :w],
                                 start=True, stop=True)
                nc.scalar.activation(x1T[:, c * TW:c * TW + w], ps2[:, :w],
                                     Act.Lrelu, bias=bias_sb[:, 1:2],
                                     alpha=SLOPE)
                transpose_store(x1T[:, c * TW:c * TW + w], c, w, y_loc[0])
                if c in AG_AFTER_TILE:
                    ag_chunk(AG_AFTER_TILE[c], y_loc[0], xf[0])

            # ================= RGCN layers =================
            # queue tracks Tile's global DMASW lane rotation: gathers are
            # the ONLY Pool-engine DMA insts (collectives ride the
            # dedicated Collectives proc), so a single global counter
            # keeps lane%4 == queue for every gather.
            qctr = [0]

            def rgcn_layer(xfull, x_curT, is_last, yl, xfn, yname):
                yT = big.tile([128, L], f16, tag="bigT", name=yname)
                half_base = [xfull[0:SPLIT, :], xfull[SPLIT:NPAD, :]]
                agg = [None] * R     # psum accumulators for current tile
                accT = [None] * R    # drained SBUF f16 accs

                def finish_tile(t):
                    w = _tile_w(t)
                    ops = psp.tile([128, TW], f32, tag="out")
                    nc.tensor.matmul(ops[:, :w], wroot_sb[:],
                                     x_curT[:, t * TW:t * TW + w],
                                     start=True, stop=False)
                    for ri in range(R):
                        nc.tensor.matmul(ops[:, :w], wrel_sb[:, ri, :],
                                         accT[ri][:, :w],
                                         start=False, stop=(ri == R - 1))
                    nc.vector.tensor_scalar(
                        out=yT[:, t * TW:t * TW + w], in0=ops[:, :w],
                        scalar1=bias_sb[:, 2:3], scalar2=None, op0=Alu.add)
                    if not is_last:
                        transpose_store(yT[:, t * TW:t * TW + w], t, w, yl)
                        if t in AG_AFTER_TILE:
                            ag_chunk(AG_AFTER_TILE[t], yl, xfn)

                blk_i = 0
                for (h, s0, ntok) in chunks:
                    nb = ntok // 128
                    g = wk.tile([128, CHUNK // 128, D], f16, tag="g", bufs=12)
                    nc.gpsimd.dma_gather(
                        out_ap=g[:, :nb, :],
                        in_ap=half_base[h],
                        idxs_ap=gidx_sb[:, s0 // 16:(s0 + ntok) // 16],
                        num_idxs=ntok,
                        num_idxs_reg=ntok,
                        elem_size=D,
                        queue_num=qctr[0] % NQUEUES,
                    )
                    qctr[0] += 1
                    for j in range(nb):
                        bh, bt, br, first, last = sched[blk_i]
                        assert bh == h
                        w = _tile_w(bt)
                        col = s0 // 128 + j
                        ind = wk.tile([128, TW], f16, tag="ind", bufs=16)
                        nc.vector.tensor_scalar(
                            out=ind[:, :w], in0=iota_sb[:, :w],
                            scalar1=gdst_sb[:, col:col + 1],
                            scalar2=grcp_sb[:, col:col + 1],
                            op0=Alu.is_equal, op1=Alu.mult)
                        if first:
                            agg[br] = psp.tile([128, TW], f32,
                                               tag=f"agg{br}", bufs=1,
                                               name=f"agg{br}")
                        nc.tensor.matmul(agg[br][:, :w], g[:, j, :],
                                         ind[:, :w], start=first, stop=last)
                        if last:
                            acc = wk.tile([128, TW], f16, tag="accT", bufs=8)
                            nc.scalar.activation(acc[:, :w], agg[br][:, :w],
                                                 Act.Copy)
                            accT[br] = acc
                            if br == R - 1:
                                finish_tile(bt)
                        blk_i += 1
                assert blk_i == len(sched)
                return yT

            y1T = rgcn_layer(xf[0], x1T, False, y_loc[1], xf[1], "y1T")
            y2T = rgcn_layer(xf[1], y1T, True, None, None, "y2T")

            # ================= out MLP =================
            for c in range(NT):
                w = _tile_w(c)
                ps = psp.tile([128, TW], f32, tag="out")
                nc.tensor.matmul(ps[:, :w], wout1_sb[:],
                                 y2T[:, c * TW:c * TW + w],
                                 start=True, stop=True)
                z1 = wk.tile([128, TW], f16, tag="x0c")
                nc.scalar.activation(z1[:, :w], ps[:, :w], Act.Lrelu,
                                     bias=bias_sb[:, 3:4], alpha=SLOPE)
                ps2 = psp.tile([2, TW], f32, tag="out", name="ps2o")
                nc.tensor.matmul(ps2[:, :w], wout2_sb[:], z1[:, :w],
                                 start=True, stop=True)
                oc = wk.tile([2, TW], f32, tag="oc", bufs=2)
                nc.vector.tensor_scalar(
                    out=oc[:, :w], in0=ps2[:, :w],
                    scalar1=bout2_sb[:, 0:1], scalar2=None, op0=Alu.add)
                nc.sync.dma_start(out=out_d[:, c * TW:c * TW + w],
                                  in_=oc[:, :w])

    nc.compile()
    return nc


def kernel(des, tweet, num_prop, cat_prop, edge_index, edge_type,
           W_des, b_des, W_in, b_in, W_rel, W_root, b_rgcn,
           W_out1, b_out1, W_out2, b_out2):
    import time
    from concourse.bass_utils import run_bass_kernel_spmd

    gidx_w, gdst_w, grcp_w, sched, chunks, TTOT, NBLK = _prep_edges(
        np.asarray(edge_index), np.asarray(edge_type))

    t0 = time.time()
    nc = _build(sched, chunks, TTOT, NBLK)
    t1 = time.time()

    des_pad = np.zeros((NPAD, DDES), np.float16)
    des_pad[:N] = np.asarray(des, np.float32).astype(np.float16)
    bias = np.stack([np.asarray(b_des, np.float32),
                     np.asarray(b_in, np.float32),
                     np.asarray(b_rgcn, np.float32),
                     np.asarray(b_out1, np.float32)], axis=1)  # [128,4]
    common = {
        "wdes": np.asarray(W_des, np.float32).astype(np.float16),
        "win": np.asarray(W_in, np.float32).astype(np.float16),
        "wroot": np.asarray(W_root, np.float32).astype(np.float16),
        "wrel": np.asarray(W_rel, np.float32).astype(np.float16),
        "wout1": np.asarray(W_out1, np.float32).astype(np.float16),
        "wout2": np.asarray(W_out2, np.float32).astype(np.float16),
        "bias": bias,
        "bout2": np.asarray(b_out2, np.float32).reshape(2, 1),
    }
    in_maps = []
    for m in range(M):
        dshard = des_pad[m * L:(m + 1) * L].T       # [768, L]
        desT6 = np.ascontiguousarray(
            dshard.reshape(6, 128, L).transpose(1, 0, 2))  # [128, 6, L]
        in_maps.append({
            "desT": desT6,
            "gidx": gidx_w[m], "gdst": gdst_w[m], "grcp": grcp_w[m],
            **common,
        })

    trace = bool(_LAST.get("trace"))
    res = run_bass_kernel_spmd(nc, in_maps, list(range(M)), trace=trace)
    t2 = time.time()
    _LAST["build_s"] = t1 - t0
    _LAST["run_s"] = t2 - t1
    _LAST["exec_ns"] = res.exec_time_ns
    _LAST["ttot"] = TTOT
    _LAST["res"] = res

    out = np.concatenate([res.results[m]["out"].T for m in range(M)], axis=0)
    return np.ascontiguousarray(out[:N].astype(np.float32))
